# revision 12
# baseline (speedup 1.0000x reference)
"""GAT layer (nn_SPACY_GraphAttentionLayer) Trainium2 Bass kernel.

Data-parallel over batch: 8 graphs -> 8 NeuronCores, one graph per core.

Math (per graph):
  Wh = h @ W1, Wj = j @ W2, V = Wh + Wj
  Wh1_i = (Wh @ a1)_i,  Wh2_k = (Wh @ a2)_k
  z[i,k] = Wh1_i + Wh2_k
  logits = where(adj>0, leaky_relu(z, 0.2), -inf)
  A = softmax(logits, axis k);  out = elu(A @ V)

Factorizations used on-chip:
  exp(lrelu(z)) = exp(0.8*relu(z)) * exp(0.2*Wh1_i) * exp(0.2*Wh2_k)
  The exp(0.2*Wh1_i) row factor cancels in the softmax.  Since exp is
  monotone, exp(0.8*relu(z)) = max(exp(0.8*z), 1) = max(u_i * v_k, 1)
  with u = exp(0.8*Wh1), v = exp(0.8*Wh2) -- so the NxN activation
  becomes a single 4x-mode tensor_scalar on DVE.
  With q = max(u_i*v_k, 1), p = adj * q, e_k = exp(0.2*Wh2_k):
     A[i,k] = p[i,k]*e_k / sum_k p[i,k]*e_k
  The rhs is built as [(V-1)*e | e] so the matmul yields
  num' = num - den and x-1 = num'*rs directly; then
     elu(x) = max(x,0)-1 + min(exp(x),1)
            = tensor_scalar(ops,rs,-1,mult,max) + min(exp(rs*ops+1),1).

h/j are uploaded pre-transposed (hT/jT, a host-side layout change of the
same bytes) so the f-contraction needs no on-chip transposes.

Engine budget per row-block (~2.2us cadence):
  DVE:  q (4x tensor_scalar) + p = q*adj (2x TT) + small pt slice
  Act:  most of the pt PSUM->SBUF copies + exp epilogue
  PE:   16 transposes + 16 accumulating matmuls
  Pool: h/j + adj cast-DMA triggers, v0/fin epilogue
  sync HWDGE: weights + out stores
"""
import sys
import numpy as np

sys.path.insert(0, "/opt/trn_rl_repo")

B, N, F, O = 8, 2048, 256, 128
P = 128
NB = N // P  # 16 row/col chunks

_CACHE = {}
VARIANT = {"apre": 6, "pso": 3, "pst": 3, "hipri": True, "hipri2": True,
           "epi_pool": True, "pt_dve": 0}


def _build_nc(repeat=1, loop_iters=1):
    import ml_dtypes
    from contextlib import ExitStack
    import concourse.bass as bass
    import concourse.tile as tile
    from concourse import bacc, mybir

    f32 = mybir.dt.float32
    bf16 = mybir.dt.bfloat16
    fp16 = mybir.dt.float16
    i32 = mybir.dt.int32
    Alu = mybir.AluOpType
    Act = mybir.ActivationFunctionType

    nc = bacc.Bacc()
    hT_d = nc.dram_tensor("hT", [F, N], f32, kind="ExternalInput")
    jT_d = nc.dram_tensor("jT", [F, N], f32, kind="ExternalInput")
    adj_d = nc.dram_tensor("adj", [N, N], i32, kind="ExternalInput")
    W1_d = nc.dram_tensor("W1", [F, O], f32, kind="ExternalInput")
    W2_d = nc.dram_tensor("W2", [F, O], f32, kind="ExternalInput")
    a_d = nc.dram_tensor("a", [2 * O, 1], f32, kind="ExternalInput")
    out_d = nc.dram_tensor("out", [N, O], f32, kind="ExternalOutput")

    identb_d = nc.inline_tensor(np.eye(P, dtype=ml_dtypes.bfloat16), name="identb")
    identh_d = nc.inline_tensor(np.eye(P, dtype=np.float16), name="identh")

    with tile.TileContext(nc) as tc, ExitStack() as ctx:
        cpool = ctx.enter_context(tc.tile_pool(name="cpool", bufs=1))
        wpool = ctx.enter_context(tc.tile_pool(name="wpool", bufs=1))
        bpool = ctx.enter_context(tc.tile_pool(name="bpool", bufs=3))
        apool = ctx.enter_context(tc.tile_pool(name="apool", bufs=VARIANT.get("apre", 6)))
        qpool = ctx.enter_context(tc.tile_pool(name="qpool", bufs=6))
        spool = ctx.enter_context(tc.tile_pool(name="spool", bufs=2))
        psM = ctx.enter_context(tc.tile_pool(name="psM", bufs=1, space="PSUM"))
        psT = ctx.enter_context(tc.tile_pool(name="psT", bufs=VARIANT.get("pst", 3), space="PSUM"))
        psR = ctx.enter_context(tc.tile_pool(name="psR", bufs=1, space="PSUM"))
        psO = ctx.enter_context(tc.tile_pool(name="psO", bufs=VARIANT.get("pso", 3), space="PSUM"))

        # ---------------- Stage A: weights prep ----------------
        identb = cpool.tile([P, P], bf16, tag="identb")
        nc.sync.dma_start(identb[:], identb_d[:])
        identh = cpool.tile([P, P], fp16, tag="identh")
        nc.sync.dma_start(identh[:], identh_d[:])
        ones1 = cpool.tile([1, P], f32, tag="ones1")
        nc.vector.memset(ones1[:], 1.0)

        w1s = wpool.tile([P, 2, O], f32, tag="w1s")
        nc.sync.dma_start(w1s[:], W1_d.rearrange("(c p) o -> p c o", p=P))
        w2s = wpool.tile([P, 2, O], f32, tag="w2s")
        nc.sync.dma_start(w2s[:], W2_d.rearrange("(c p) o -> p c o", p=P))
        a12 = wpool.tile([P, 2], f32, tag="a12")
        nc.sync.dma_start(a12[:], a_d.rearrange("(c p) one -> p (c one)", p=P))

        w1b = wpool.tile([P, 2, O], fp16, tag="w1b")
        nc.vector.tensor_copy(w1b[:], w1s[:])
        w2b = wpool.tile([P, 2, O], fp16, tag="w2b")
        nc.vector.tensor_copy(w2b[:], w2s[:])
        a12b = wpool.tile([P, 2], fp16, tag="a12b")
        nc.vector.tensor_copy(a12b[:], a12[:])

        # W1^T chunks (fp16) then w1a | w2a = W1 @ [a1 | a2]
        w1t_ps = psM.tile([P, 2, P], fp16, tag="psM")
        for c in range(2):
            nc.tensor.transpose(w1t_ps[:, c, :], w1b[:, c, :], identh[:])
        w1t = wpool.tile([P, 2, P], fp16, tag="w1t")
        nc.vector.tensor_copy(w1t[:], w1t_ps[:])
        wab_ps = psM.tile([P, 2, 2], f32, tag="psM")
        for c in range(2):
            nc.tensor.matmul(wab_ps[:, c, :], w1t[:, c, :], a12b[:], start=True, stop=True)
        wab = wpool.tile([P, 2, 2], fp16, tag="wab")
        nc.vector.tensor_copy(wab[:], wab_ps[:])

        # rhs for projections: r1 = [W1 | w1a | w2a], r2z = [W2 | 0 | 0]
        r1 = wpool.tile([P, 2, 130], fp16, tag="r1")
        nc.vector.tensor_copy(r1[:, :, 0:128], w1b[:])
        nc.vector.tensor_copy(r1[:, :, 128:130], wab[:])
        r2z = wpool.tile([P, 2, 130], fp16, tag="r2z")
        nc.vector.memset(r2z[:], 0.0)
        nc.vector.tensor_copy(r2z[:, :, 0:128], w2b[:])

        def body():
            for _rep in range(repeat):
                stage_bc(nc, tc, mybir, Alu, Act,
                         cpool, wpool, bpool, apool, qpool, spool, psM, psT, psO, psR,
                         hT_d, jT_d, adj_d, out_d,
                         identb, identh, ones1, r1, r2z, w2b, wab)
        if loop_iters > 1:
            ET = mybir.EngineType
            with tc.For_i(0, loop_iters, 1,
                          hint_engines=(ET.PE, ET.DVE, ET.Activation, ET.SP)):
                body()
        else:
            body()

    nc.finalize()
    return nc


def stage_bc(nc, tc, mybir, Alu, Act,
             cpool, wpool, bpool, apool, qpool, spool, psM, psT, psO, psR,
             hT_d, jT_d, adj_d, out_d,
             identb, identh, ones1, r1, r2z, w2b, wab):
    f32 = mybir.dt.float32
    bf16 = mybir.dt.bfloat16
    fp16 = mybir.dt.float16

    # ------- Stage B: load pre-transposed h/j via cast DMA, project -------
    hT_r = hT_d.rearrange("(c p) n -> p c n", p=P)   # c: f-chunk
    jT_r = jT_d.rearrange("(c p) n -> p c n", p=P)
    hTs = bpool.tile([P, 2, N], fp16, tag="hTs")
    jTs = bpool.tile([P, 2, N], fp16, tag="jTs")
    HN = N // 2
    for half in range(2):
        nsl = slice(half * HN, (half + 1) * HN)
        if VARIANT.get("hipri2"):
            with tc.high_priority():
                nc.gpsimd.dma_start(hTs[:, :, nsl], hT_r[:, :, nsl])
                nc.gpsimd.dma_start(jTs[:, :, nsl], jT_r[:, :, nsl])
        else:
            nc.gpsimd.dma_start(hTs[:, :, nsl], hT_r[:, :, nsl])
            nc.gpsimd.dma_start(jTs[:, :, nsl], jT_r[:, :, nsl])

    ub = bpool.tile([P, NB], f32, tag="ub")      # u = exp(0.8*Wh1) per row-block col
    ecf = bpool.tile([P, NB], f32, tag="ecf")    # e = exp(0.2*Wh2) per chunk
    rhs_att = bpool.tile([P, NB, 129], fp16, tag="rhs_att")  # [(V-1)*e | e] per chunk
    vbc = bpool.tile([P, N], bf16, tag="vbc")    # v = exp(0.8*Wh2) bcast down partitions

    rowps = None
    for c16 in range(NB):
        g = c16 // 4
        csl = slice(c16 * P, (c16 + 1) * P)
        if c16 % 4 == 0:
            rowps = psR.tile([1, 512], f32, tag="psR")
            gsl = slice(g * 512, (g + 1) * 512)
            # Wh2 row segment via M=1 matvec over the whole group
            nc.tensor.matmul(rowps[0:1, :], wab[:, 0, 1:2],
                             hTs[:, 0, gsl], start=True, stop=False)
            nc.tensor.matmul(rowps[0:1, :], wab[:, 1, 1:2],
                             hTs[:, 1, gsl], start=False, stop=True)
        psv = psO.tile([P, 130], f32, tag="psO")
        nc.tensor.matmul(psv[:], hTs[:, 0, csl], r1[:, 0, :], start=True, stop=False)
        nc.tensor.matmul(psv[:], hTs[:, 1, csl], r1[:, 1, :], start=False, stop=False)
        nc.tensor.matmul(psv[:], jTs[:, 0, csl], r2z[:, 0, :], start=False, stop=False)
        nc.tensor.matmul(psv[:], jTs[:, 1, csl], r2z[:, 1, :], start=False, stop=True)
        nc.scalar.activation(ub[:, c16 : c16 + 1], psv[:, 128:129], Act.Exp, scale=0.8)
        nc.scalar.activation(ecf[:, c16 : c16 + 1], psv[:, 129:130], Act.Exp, scale=0.2)
        # rhs chunk = (V - 1) * e_k, fp16; plus e column at position 128
        nc.vector.tensor_scalar(
            rhs_att[:, c16, 0:128], psv[:, 0:128], 1.0, ecf[:, c16 : c16 + 1],
            Alu.subtract, Alu.mult
        )
        nc.vector.tensor_copy(rhs_att[:, c16, 128:129], ecf[:, c16 : c16 + 1])
        if c16 % 4 == 3:
            # broadcast this 512-wide Wh2 segment down all partitions, exp'd
            rowsb = spool.tile([1, 512], f32, tag="rowsb")
            nc.vector.tensor_copy(rowsb[:], rowps[:])
            psbc = psM.tile([P, 512], f32, tag="psM")
            nc.tensor.matmul(psbc[:], ones1[:], rowsb[:], start=True, stop=True)
            nc.scalar.activation(vbc[:, g * 512 : (g + 1) * 512], psbc[:],
                                 Act.Exp, scale=0.8)

    # ---------------- Stage C: attention row-blocks ----------------
    out_r = out_d.rearrange("(rb p) o -> p rb o", p=P)
    nde = VARIANT.get("pt_dve", 4)  # pt chunks copied by DVE (rest on Act)
    _adjkeep = [None]
    for rb in range(NB):
        if VARIANT.get("one_adj"):
            if rb == 0:
                adjb = apool.tile([P, N], bf16, tag="adjb")
                nc.gpsimd.dma_start(adjb[:], adj_d[rb * P : (rb + 1) * P, :])
                _adjkeep[0] = adjb
            else:
                adjb = _adjkeep[0]
        elif VARIANT.get("hipri"):
            adjb = apool.tile([P, N], bf16, tag="adjb")
            with tc.high_priority():
                nc.gpsimd.dma_start(adjb[:], adj_d[rb * P : (rb + 1) * P, :])
        else:
            adjb = apool.tile([P, N], bf16, tag="adjb")
            nc.gpsimd.dma_start(adjb[:], adj_d[rb * P : (rb + 1) * P, :])

        pt = qpool.tile([P, NB, P], bf16, tag="pt")
        q = qpool.tile([P, N], bf16, tag="q")
        p = qpool.tile([P, N], bf16, tag="p")
        for hh in range(2):
            sl = slice(hh * HN, (hh + 1) * HN)
            # q = max(u_i * v_k, 1)
            nc.vector.tensor_scalar(
                q[:, sl], vbc[:, sl], ub[:, rb : rb + 1], 1.0,
                Alu.mult, Alu.max)
            nc.vector.tensor_tensor(p[:, sl], q[:, sl], adjb[:, sl], Alu.mult)
            tps = psT.tile([P, 8, P], bf16, tag="psT")
            for c8 in range(8):
                cc = hh * 8 + c8
                nc.tensor.transpose(
                    tps[:, c8, :], p[:, cc * P : (cc + 1) * P], identb[:])
            base = hh * 8
            ndve = max(0, min(8, nde - base)) if nde > base else 0
            if ndve > 0:
                nc.vector.tensor_scalar(
                    pt[:, base : base + ndve, :], tps[:, 0:ndve, :], 1.0, None,
                    Alu.mult)
            if ndve < 8:
                nc.scalar.copy(pt[:, base + ndve : base + 8, :], tps[:, ndve:8, :])

        ops = psO.tile([P, 130], f32, tag="psO")
        for c in range(NB):
            nc.tensor.matmul(
                ops[:, 0:129],
                pt[:, c, :],
                rhs_att[:, c, :],
                start=(c == 0),
                stop=(c == NB - 1),
            )

        # epilogue: x-1 = ops*rs ; elu(x) = max(x,0)-1 + min(exp(x),1)
        rs = spool.tile([P, 1], f32, tag="rs")
        nc.vector.reciprocal(rs[:], ops[:, 128:129])
        v0 = spool.tile([P, O], f32, tag="v0")
        e2 = spool.tile([P, O], f32, tag="e2")
        fin = spool.tile([P, O], f32, tag="fin")
        if VARIANT.get("epi_pool"):
            nc.gpsimd.tensor_scalar(v0[:], ops[:, 0:128], rs[:], -1.0, Alu.mult, Alu.max)
            nc.scalar.activation(e2[:], ops[:, 0:128], Act.Exp, scale=rs[:], bias=1.0)
            nc.vector.scalar_tensor_tensor(fin[:], e2[:], 1.0, v0[:], Alu.min, Alu.add)
        else:
            nc.vector.tensor_scalar(v0[:], ops[:, 0:128], rs[:], -1.0, Alu.mult, Alu.max)
            nc.scalar.activation(e2[:], ops[:, 0:128], Act.Exp, scale=rs[:], bias=1.0)
            nc.vector.scalar_tensor_tensor(fin[:], e2[:], 1.0, v0[:], Alu.min, Alu.add)
        nc.sync.dma_start(out_r[:, rb, :], fin[:])


def get_nc(repeat=1, loop_iters=1):
    key = ("nc", repeat, loop_iters, tuple(sorted(VARIANT.items())))
    if key not in _CACHE:
        _CACHE[key] = _build_nc(repeat, loop_iters)
    return _CACHE[key]


def prep_core_inputs(h_b, j_b, adj_b, W1, W2, a):
    """Host-side input staging for one graph: h/j are uploaded transposed."""
    return {
        "hT": np.ascontiguousarray(np.asarray(h_b, np.float32).T),
        "jT": np.ascontiguousarray(np.asarray(j_b, np.float32).T),
        "adj": np.ascontiguousarray(adj_b),
        "W1": np.ascontiguousarray(W1),
        "W2": np.ascontiguousarray(W2),
        "a": np.ascontiguousarray(a),
    }


def run(h, j, adj, W1, W2, a, trace=False):
    from concourse.bass_utils import run_bass_kernel_spmd

    nc = get_nc()
    in_maps = [prep_core_inputs(h[b], j[b], adj[b], W1, W2, a) for b in range(B)]
    res = run_bass_kernel_spmd(nc, in_maps, core_ids=list(range(B)), trace=trace)
    out = np.stack([res.results[b]["out"] for b in range(B)], axis=0)
    return out, res


def kernel(h, j, adj, W1, W2, a):
    h = np.asarray(h, dtype=np.float32)
    j = np.asarray(j, dtype=np.float32)
    adj = np.asarray(adj, dtype=np.int32)
    W1 = np.asarray(W1, dtype=np.float32)
    W2 = np.asarray(W2, dtype=np.float32)
    a = np.asarray(a, dtype=np.float32)
    out, _ = run(h, j, adj, W1, W2, a, trace=False)
    return out
